# revision 16
# baseline (speedup 1.0000x reference)
"""Trainium2 Bass kernel for BatchedLUTNodes (v2, fp16).

Math: out[b,n] = sum_e tables[n,e] * prod_i (x_i*bit_i(e) + (1-x_i)*(1-bit_i(e)))
is 6-dim multilinear interpolation. In the monomial basis:
    out[b,n] = sum_{p,k} u_p[b,n] * C[n][p,k] * v_k[b,n]
with u = monomials of (x0,x1,x2) and v = monomials of (x3,x4,x5), each 8-wide
in slot order [1, a, b, c, ab, ac, bc, abc]; C[n] (8x8) is the Moebius
(finite-difference) transform of tables[n], computed on the host.

Per core (1024 nodes, batch=128 on partitions, 8 node-tiles of 128 nodes):
  - v arrives PRE-TRANSPOSED from the host (vt: [(g,k) partitions, b] per
    16-node group) so no PE transposes are needed; everything is fp16.
  - stage 1: per (tile, group-of-16) one 128x128 fp16 matmul of vt against a
    block-diagonal C operand (one zeroed [128,8192] fp16 SBUF region + 16
    scatter-DMAs) -> y[b, (G, p, g)] in PSUM fp32.
  - ACT evacuates y PSUM->SBUF fp16 (enables DVE 2x mode downstream).
  - u is built in SBUF [b, (p, node)] p-major: raw x0..x2 DMA'd straight into
    slots 1..3, slot 0 memset to ones, slots 4..7 via three DVE products.
  - z = y * u on DVE (all-fp16, 2x), then a 3-level binary add tree over p
    (2x) with the last level writing dense out columns; Pool takes the first
    tree level on alternating tiles to balance engines.

Engine waits are kept within the walrus ~2-sync-wait limit via same-engine
`chain` edges plus the `_split_multiwait` carrier pass (from the v1 kernel).

Sharding: nodes split 8 ways (1024/core), C sharded alongside.
"""

import numpy as np
from contextlib import ExitStack

try:
    from concourse import bass, tile
    from concourse import bass_utils
except ImportError:
    import sys
    sys.path.insert(0, "/opt/trn_rl_repo")
    from concourse import bass, tile
    from concourse import bass_utils

from concourse import masks
from concourse.tile import add_dep_helper

mybir = bass.mybir
F32 = mybir.dt.float32
F16 = mybir.dt.float16

B = 128            # batch (partition dim)
N = 8192           # total nodes
NCORES = 8
NPC = N // NCORES  # nodes per core = 1024
NT = 8             # node-tiles per core (128 nodes each)
TN = 128           # nodes per tile
NG = 8             # matmul groups per tile
GN = 16            # nodes per group
R2 = NT * 1024     # flat row length of the big [128, 8192] SBUF regions


def build_nc(dbg: bool = False) -> bass.Bass:
    nc = bass.Bass("TRN2", target_bir_lowering=False, debug=False)
    # ux: raw u-vars, p-major: col (p-1)*1024 + nl holds x_p-var[b, node nl]
    ux = nc.dram_tensor("ux", [B, 3 * NPC], F16, kind="ExternalInput")
    # vt: host-transposed v monomials: vt[8g+k, t*1024 + G*128 + b]
    vt = nc.dram_tensor("vt", [128, R2], F16, kind="ExternalInput")
    # cden: C in (g, k, p, G, t) order -> 512-elem runs per (g,k)
    cden = nc.dram_tensor("cden", [GN * 8 * 512], F16, kind="ExternalInput")
    out = nc.dram_tensor("out", [B, NPC], F16, kind="ExternalOutput")
    if dbg:
        d_u = nc.dram_tensor("d_u", [B, 8 * NPC], F16, kind="ExternalOutput")
        d_cd = nc.dram_tensor("d_cd", [128, R2], F16, kind="ExternalOutput")
        d_vt = nc.dram_tensor("d_vt", [128, R2], F16, kind="ExternalOutput")
        d_y = nc.dram_tensor("d_y", [B, 1024], F32, kind="ExternalOutput")
        d_ysb = nc.dram_tensor("d_ysb", [B, 1024], F16, kind="ExternalOutput")
        d_z = nc.dram_tensor("d_z", [B, 1024], F16, kind="ExternalOutput")
        d_zt = nc.dram_tensor("d_zt", [B, 1024], F16, kind="ExternalOutput")
        d_o = nc.dram_tensor("d_o", [B, 128], F16, kind="ExternalOutput")

    chain_prev = {}

    def chain(key, binst):
        # same-engine program-order edge: no semaphore cost, but keeps
        # the scheduler from reordering so sem-wait elision works
        prev = chain_prev.get(key)
        if prev is not None:
            add_dep_helper(binst.ins, prev, sync=False, reason=f"{key} order chain")
        chain_prev[key] = binst.ins
        return binst

    # scratch sems for the multi-wait splitting pass
    wsems = {e: nc.alloc_semaphore(f"wsplit_{e.name}")
             for e in (mybir.EngineType.Pool, mybir.EngineType.Activation,
                       mybir.EngineType.PE, mybir.EngineType.DVE,
                       mybir.EngineType.SP)}
    nc._wsplit_sems = wsems
    nc._wsplit_clears = []

    with tile.TileContext(nc) as tc:
        with ExitStack() as ctx:
            for eng, h in ((nc.gpsimd, wsems[mybir.EngineType.Pool]),
                           (nc.scalar, wsems[mybir.EngineType.Activation]),
                           (nc.tensor, wsems[mybir.EngineType.PE]),
                           (nc.vector, wsems[mybir.EngineType.DVE]),
                           (nc.sync, wsems[mybir.EngineType.SP])):
                nc._wsplit_clears.append(eng.sem_clear(h).ins)
            consts = ctx.enter_context(tc.tile_pool(name="consts", bufs=1))
            ypool = ctx.enter_context(tc.tile_pool(name="ysb", bufs=2))
            zpool = ctx.enter_context(tc.tile_pool(name="z", bufs=2))
            opool = ctx.enter_context(tc.tile_pool(name="o", bufs=1))
            y_psum = ctx.enter_context(tc.tile_pool(name="y", bufs=3, space="PSUM"))

            ident = consts.tile([128, 128], F32)
            masks.make_identity(nc, ident[:])

            # carrier templates for the multi-wait split pass
            cps = ctx.enter_context(tc.tile_pool(name="cps", bufs=1, space="PSUM"))
            cps_t = cps.tile([128, 512], F32)
            scrP = consts.tile([128, 4], F32, tag="scrP")
            scrD = consts.tile([128, 4], F32, tag="scrD")
            scrA = consts.tile([128, 4], F32, tag="scrA")
            tpl = {}
            tpl[mybir.EngineType.Pool] = nc.gpsimd.memset(scrP[:], 0.0).ins
            tpl[mybir.EngineType.DVE] = nc.vector.memset(scrD[:], 0.0).ins
            tpl[mybir.EngineType.Activation] = nc.scalar.copy(
                scrA[:], ident[:, 0:4]).ins
            tpl[mybir.EngineType.PE] = nc.tensor.transpose(
                cps_t[:, 0:128], ident[:], ident[:]).ins
            # SP carrier: a scratch-sem clear — same-engine program order is
            # what makes a hoisted wait actually gate the instruction
            tpl[mybir.EngineType.SP] = nc.sync.sem_clear(
                wsems[mybir.EngineType.SP]).ins
            nc._wsplit_tpl = tpl

            # ---- persistent SBUF regions -------------------------------
            U = nc.alloc_sbuf_tensor("u_all", [128, 8 * NPC], F16)
            cd = nc.alloc_sbuf_tensor("cd_all", [128, R2], F16)
            vts = nc.alloc_sbuf_tensor("vt_all", [128, R2], F16)

            # input DMAs (HW DGE queues, non-Pool engines)
            nc.sync.dma_start(
                bass.AP(U, 1 * NPC, [[8 * NPC, 128], [1, 3 * NPC]]),
                ux[:, :])
            for t in range(NT):
                nc.scalar.dma_start(vts[:, t * 1024:(t + 1) * 1024],
                                    vt[:, t * 1024:(t + 1) * 1024])

            # cd zero-fill split across three engines, then scatter C in
            chain('DVE', nc.vector.memset(cd[:, 0:2816], 0.0))
            chain('ACT', nc.scalar.memzero(cd[:, 2816:5632]))
            chain('POOL', nc.gpsimd.memset(cd[:, 5632:R2], 0.0))
            # scatter C diagonal runs: cd[(g,k), (p*16+g)*64 + (G*8+t)]
            # (64-elem runs keep the matmul rhs a single-level stride-64 AP)
            for g in range(GN):
                dst = bass.AP(cd, 8 * g * R2 + g * 64,
                              [[R2, 8], [1024, 8], [1, 64]])
                src = bass.AP(cden, g * 4096, [[512, 8], [64, 8], [1, 64]])
                eng = nc.sync if g % 2 == 0 else nc.scalar
                eng.dma_start(dst, src)

            # u: ones slot + product slots 4..7 (p-major layout)
            chain('POOL', nc.gpsimd.memset(
                bass.AP(U, 0, [[8 * NPC, 128], [1, NPC]]), 1.0))
            uap = lambda p, d=1: bass.AP(
                U, p * NPC, [[8 * NPC, 128], [NPC, d], [1, NPC]])
            chain('DVE', nc.vector.tensor_tensor(
                uap(4), uap(1), uap(2), mybir.AluOpType.mult))
            chain('DVE', nc.vector.tensor_tensor(
                uap(5, 2), uap(1, 2),
                bass.AP(U, 3 * NPC, [[8 * NPC, 128], [0, 2], [1, NPC]]),
                mybir.AluOpType.mult))
            chain('DVE', nc.vector.tensor_tensor(
                uap(7), uap(4), uap(3), mybir.AluOpType.mult))

            out_sb = opool.tile([128, NPC], F16)

            if dbg:
                nc.sync.dma_start(d_u[:, :], bass.AP(
                    U, 0, [[8 * NPC, 128], [1, 8 * NPC]]))
                nc.sync.dma_start(d_cd[:, :], cd[:, :])
                nc.sync.dma_start(d_vt[:, :], vts[:, :])

            for t in range(NT):
                yp = y_psum.tile([128, 1024], F32, tag="yp")
                for G in range(NG):
                    rhs = bass.AP(cd, G * 8 + t,
                                  [[R2, 128], [64, 128]])
                    chain('PE', nc.tensor.matmul(
                        yp[:, G * 128:(G + 1) * 128],
                        lhsT=vts[:, t * 1024 + G * 128:t * 1024 + (G + 1) * 128],
                        rhs=rhs,
                        start=True, stop=True,
                    ))

                # evacuate PSUM -> SBUF fp16 on ACT (enables DVE 2x)
                ysb = ypool.tile([128, 1024], F16, tag="ysb")
                chain('ACT', nc.scalar.copy(ysb[:], yp[:]))
                if dbg and t == 0:
                    nc.sync.dma_start(d_ysb[:, :], ysb[:])

                # z = y * u  (all fp16 SBUF -> 2x)
                z = zpool.tile([128, 1024], F16, tag="z")
                uin = bass.AP(U, t * TN,
                              [[8 * NPC, 128], [GN, NG], [NPC, 8], [1, GN]])
                chain('DVE', nc.vector.tensor_tensor(
                    z[:].rearrange("a (G p g) -> a G p g", p=8, g=GN),
                    ysb[:].rearrange("a (G p g) -> a G p g", p=8, g=GN),
                    uin, mybir.AluOpType.mult))

                if dbg and t == 0:
                    nc.sync.dma_start(d_z[:, :], z[:])

                # binary add tree over p; L1 alternates DVE/Pool
                z4 = z[:].rearrange("a (G p g) -> a G p g", p=8, g=GN)
                l1eng, l1key = ((nc.gpsimd, 'POOL') if t % 2 == 0
                                else (nc.vector, 'DVE'))
                chain(l1key, l1eng.tensor_tensor(
                    z4[:, :, 0:4], z4[:, :, 0:4], z4[:, :, 4:8],
                    mybir.AluOpType.add))
                chain('DVE', nc.vector.tensor_tensor(
                    z4[:, :, 0:2], z4[:, :, 0:2], z4[:, :, 2:4],
                    mybir.AluOpType.add))
                chain('DVE', nc.vector.tensor_tensor(
                    out_sb[:, t * TN:(t + 1) * TN]
                        .rearrange("a (G q g) -> a G q g", q=1, g=GN),
                    z4[:, :, 0:1], z4[:, :, 1:2],
                    mybir.AluOpType.add))

                if dbg and t == 0:
                    nc.sync.dma_start(d_zt[:, :], z[:])
                    nc.sync.dma_start(d_o[:, :], out_sb[:, 0:TN])

                if t % 2 == 1:
                    nc.sync.dma_start(out[:, (t - 1) * TN:(t + 1) * TN],
                                      out_sb[:, (t - 1) * TN:(t + 1) * TN])

    _split_multiwait(nc)
    return nc


def _split_multiwait(nc):
    """Hoist extra sync waits onto same-engine carrier instructions (the
    walrus codegen gives each TPB instruction ~one wait slot)."""
    import inspect
    wsems = nc._wsplit_sems
    tpl = nc._wsplit_tpl
    clears = set(id(c) for c in nc._wsplit_clears)

    sigcache = {}

    def clone(template, engine, name, w, sem):
        ty = type(template)
        if ty not in sigcache:
            sigcache[ty] = [p for p in inspect.signature(ty).parameters
                            if p not in ("name", "engine", "sync_info",
                                         "descendants", "_kwargs")]
        kw = {}
        for p in sigcache[ty]:
            if hasattr(template, p):
                v = getattr(template, p)
                if v is not None or p in ("ins", "outs"):
                    kw[p] = v
        return ty(name=name, engine=engine,
                  sync_info=mybir.SyncInfo(on_wait=[w], on_update=[]),
                  **kw)

    for fn in nc.m.functions:
        for blk in fn.blocks:
            head, out = [], []
            changed = False
            for ins in blk.instructions:
                if id(ins) in clears:
                    head.append(ins)
                    changed = True
                    continue
                si = getattr(ins, "sync_info", None)
                waits = list(si.on_wait) if si is not None else []
                if len(waits) > 1:
                    changed = True
                    eng = ins.engine
                    ceng = eng if eng in tpl else mybir.EngineType.Pool
                    # carriers are transpose clones on PE and would clobber
                    # the stationary between an Ldweights/Matmult pair —
                    # insert them BEFORE the matmul's Ldweights
                    pos = len(out)
                    if (type(ins).__name__ == "InstMatmult" and out
                            and type(out[-1]).__name__ == "InstLdweights"
                            and out[-1].engine == ins.engine):
                        pos -= 1
                    carriers = [clone(tpl[ceng], ceng,
                                      f"{ins.name}-w{i}", w, wsems[ceng])
                                for i, w in enumerate(waits[:-1])]
                    out[pos:pos] = carriers
                    ins.sync_info = mybir.SyncInfo(
                        on_wait=[waits[-1]], on_update=list(si.on_update))
                out.append(ins)
            if changed:
                blk.instructions = head + out


# ---------------------------------------------------------------- host side

# slot order [1, a, b, c, ab, ac, bc, abc] -> monomial bitmask (bit0=a,...)
SLOT2MON = np.array([0, 1, 2, 4, 3, 5, 6, 7])


def _monomial_C(tables: np.ndarray) -> np.ndarray:
    """tables (N, 64) -> C (N, 8, 8) fp32 in slot order: C[n, p, k]."""
    c = np.asarray(tables, np.float64).reshape(-1, 2, 2, 2, 2, 2, 2)
    for ax in range(1, 7):
        lo = np.take(c, 0, axis=ax)
        hi = np.take(c, 1, axis=ax)
        c = np.stack([lo, hi - lo], axis=ax)
    # axes (n, m5, m4, m3, m2, m1, m0): flat index m5*32+...+m0
    cm = c.reshape(-1, 64)
    flat = np.zeros((8, 8), np.int64)
    for jm in range(8):
        for km in range(8):
            m0, m1, m2 = jm & 1, (jm >> 1) & 1, (jm >> 2) & 1
            m3, m4, m5 = km & 1, (km >> 1) & 1, (km >> 2) & 1
            flat[jm, km] = m5 * 32 + m4 * 16 + m3 * 8 + m2 * 4 + m1 * 2 + m0
    idx = flat[SLOT2MON][:, SLOT2MON]   # idx[p, k], slot-ordered
    return cm[:, idx].astype(np.float32)  # (N, 8, 8)


def _v_monomials(xv: np.ndarray) -> np.ndarray:
    """xv (..., 3) -> (..., 8) slot-order monomials [1,a,b,c,ab,ac,bc,abc]."""
    a, b, c = xv[..., 0], xv[..., 1], xv[..., 2]
    one = np.ones_like(a)
    return np.stack([one, a, b, c, a * b, a * c, b * c, a * b * c], axis=-1)


def make_in_maps(x: np.ndarray, tables: np.ndarray):
    x = np.clip(np.asarray(x, np.float32), 0.0, 1.0)
    C = _monomial_C(np.asarray(tables, np.float32))  # (N, 8, 8)
    in_maps = []
    for core in range(NCORES):
        sl = slice(core * NPC, (core + 1) * NPC)
        xs = x[:, sl, :]                            # (B, 1024, 6)

        # ux: [b, (j, nl)] j-major raw u-vars x0..x2
        uxc = np.ascontiguousarray(
            xs[:, :, 0:3].transpose(0, 2, 1).reshape(B, 3 * NPC)
        ).astype(np.float16)

        # vt: [8g+k, t*1024 + G*128 + b] = v_k[b, node t*128+G*16+g]
        vmon = _v_monomials(xs[:, :, 3:6]).astype(np.float16)  # (B,1024,8)
        vm = vmon.reshape(B, NT, NG, GN, 8)          # (b, t, G, g, k)
        vtc = np.ascontiguousarray(
            vm.transpose(3, 4, 1, 2, 0)              # (g, k, t, G, b)
            .reshape(128, R2))

        # cden: (g, k, p, G, t)
        Cc = C[sl].reshape(NT, NG, GN, 8, 8)         # (t, G, g, p, k)
        cdenc = np.ascontiguousarray(
            Cc.transpose(2, 4, 3, 1, 0)              # (g, k, p, G, t)
        ).reshape(GN * 8 * 512).astype(np.float16)

        in_maps.append({"ux": uxc, "vt": vtc, "cden": cdenc})
    return in_maps


_NC_CACHE = None


def _get_nc():
    global _NC_CACHE
    if _NC_CACHE is None:
        _NC_CACHE = build_nc()
    return _NC_CACHE


def kernel(x: np.ndarray, tables: np.ndarray, _trace: bool = False):
    nc = _get_nc()
    in_maps = make_in_maps(x, tables)
    res = bass_utils.run_bass_kernel_spmd(
        nc, in_maps, core_ids=list(range(NCORES)), trace=_trace,
    )
    out = np.concatenate(
        [r["out"].astype(np.float32) for r in res.results], axis=1)
    if _trace:
        return out, res
    return out


# revision 19
# speedup vs baseline: 1.0640x; 1.0640x over previous
"""Trainium2 Bass kernel for BatchedLUTNodes (v2, fp16).

Math: out[b,n] = sum_e tables[n,e] * prod_i (x_i*bit_i(e) + (1-x_i)*(1-bit_i(e)))
is 6-dim multilinear interpolation. In the monomial basis:
    out[b,n] = sum_{p,k} u_p[b,n] * C[n][p,k] * v_k[b,n]
with u = monomials of (x0,x1,x2) and v = monomials of (x3,x4,x5), each 8-wide
in slot order [1, a, b, c, ab, ac, bc, abc]; C[n] (8x8) is the Moebius
(finite-difference) transform of tables[n], computed on the host.

Per core (1024 nodes, batch=128 on partitions, 8 node-tiles of 128 nodes):
  - v arrives PRE-TRANSPOSED from the host (vt: [(g,k) partitions, b] per
    16-node group) so no PE transposes are needed; everything is fp16.
  - stage 1: per (tile, group-of-16) one 128x128 fp16 matmul of vt against a
    block-diagonal C operand (one zeroed [128,8192] fp16 SBUF region + 16
    scatter-DMAs) -> y[b, (G, p, g)] in PSUM fp32.
  - ACT evacuates y PSUM->SBUF fp16 (enables DVE 2x mode downstream).
  - u is built in SBUF [b, (p, node)] p-major: raw x0..x2 DMA'd straight into
    slots 1..3, slot 0 memset to ones, slots 4..7 via three DVE products.
  - z = y * u on DVE (all-fp16, 2x), then a 3-level binary add tree over p
    (2x) with the last level writing dense out columns; Pool takes the first
    tree level on alternating tiles to balance engines.

Engine waits are kept within the walrus ~2-sync-wait limit via same-engine
`chain` edges plus the `_split_multiwait` carrier pass (from the v1 kernel).

Sharding: nodes split 8 ways (1024/core), C sharded alongside.
"""

import numpy as np
from contextlib import ExitStack

try:
    from concourse import bass, tile
    from concourse import bass_utils
except ImportError:
    import sys
    sys.path.insert(0, "/opt/trn_rl_repo")
    from concourse import bass, tile
    from concourse import bass_utils

from concourse import masks
from concourse.tile import add_dep_helper

mybir = bass.mybir
F32 = mybir.dt.float32
F16 = mybir.dt.float16

B = 128            # batch (partition dim)
N = 8192           # total nodes
NCORES = 8
NPC = N // NCORES  # nodes per core = 1024
NT = 8             # node-tiles per core (128 nodes each)
TN = 128           # nodes per tile
NG = 8             # matmul groups per tile
GN = 16            # nodes per group
R2 = NT * 1024     # flat row length of the big [128, 8192] SBUF regions


def build_nc(dbg: bool = False) -> bass.Bass:
    nc = bass.Bass("TRN2", target_bir_lowering=False, debug=False)
    # ux: raw u-vars, p-major: col (p-1)*1024 + nl holds x_p-var[b, node nl]
    ux = nc.dram_tensor("ux", [B, 3 * NPC], F16, kind="ExternalInput")
    # vt: host-transposed v monomials: vt[8g+k, t*1024 + G*128 + b]
    vt = nc.dram_tensor("vt", [128, R2], F16, kind="ExternalInput")
    # cden: C in (g, k, p, G, t) order -> 512-elem runs per (g,k)
    cden = nc.dram_tensor("cden", [GN * 8 * 512], F16, kind="ExternalInput")
    out = nc.dram_tensor("out", [B, NPC], F16, kind="ExternalOutput")
    if dbg:
        d_u = nc.dram_tensor("d_u", [B, 8 * NPC], F16, kind="ExternalOutput")
        d_cd = nc.dram_tensor("d_cd", [128, R2], F16, kind="ExternalOutput")
        d_vt = nc.dram_tensor("d_vt", [128, R2], F16, kind="ExternalOutput")
        d_y = nc.dram_tensor("d_y", [B, 1024], F32, kind="ExternalOutput")
        d_ysb = nc.dram_tensor("d_ysb", [B, 1024], F16, kind="ExternalOutput")
        d_z = nc.dram_tensor("d_z", [B, 1024], F16, kind="ExternalOutput")
        d_zt = nc.dram_tensor("d_zt", [B, 1024], F16, kind="ExternalOutput")
        d_o = nc.dram_tensor("d_o", [B, 128], F16, kind="ExternalOutput")

    chain_prev = {}

    def chain(key, binst):
        # same-engine program-order edge: no semaphore cost, but keeps
        # the scheduler from reordering so sem-wait elision works
        prev = chain_prev.get(key)
        if prev is not None:
            add_dep_helper(binst.ins, prev, sync=False, reason=f"{key} order chain")
        chain_prev[key] = binst.ins
        return binst

    # scratch sems for the multi-wait splitting pass
    wsems = {e: nc.alloc_semaphore(f"wsplit_{e.name}")
             for e in (mybir.EngineType.Pool, mybir.EngineType.Activation,
                       mybir.EngineType.PE, mybir.EngineType.DVE,
                       mybir.EngineType.SP)}
    nc._wsplit_sems = wsems
    nc._wsplit_clears = []

    with tile.TileContext(nc) as tc:
        with ExitStack() as ctx:
            for eng, h in ((nc.gpsimd, wsems[mybir.EngineType.Pool]),
                           (nc.scalar, wsems[mybir.EngineType.Activation]),
                           (nc.tensor, wsems[mybir.EngineType.PE]),
                           (nc.vector, wsems[mybir.EngineType.DVE]),
                           (nc.sync, wsems[mybir.EngineType.SP])):
                nc._wsplit_clears.append(eng.sem_clear(h).ins)
            consts = ctx.enter_context(tc.tile_pool(name="consts", bufs=1))
            ypool = ctx.enter_context(tc.tile_pool(name="ysb", bufs=2))
            zpool = ctx.enter_context(tc.tile_pool(name="z", bufs=2))
            opool = ctx.enter_context(tc.tile_pool(name="o", bufs=1))
            y_psum = ctx.enter_context(tc.tile_pool(name="y", bufs=3, space="PSUM"))

            ident = consts.tile([128, 128], F32)
            masks.make_identity(nc, ident[:])

            # carrier templates for the multi-wait split pass
            cps = ctx.enter_context(tc.tile_pool(name="cps", bufs=1, space="PSUM"))
            cps_t = cps.tile([128, 512], F32)
            scrP = consts.tile([128, 4], F32, tag="scrP")
            scrD = consts.tile([128, 4], F32, tag="scrD")
            scrA = consts.tile([128, 4], F32, tag="scrA")
            tpl = {}
            tpl[mybir.EngineType.Pool] = nc.gpsimd.memset(scrP[:], 0.0).ins
            tpl[mybir.EngineType.DVE] = nc.vector.memset(scrD[:], 0.0).ins
            tpl[mybir.EngineType.Activation] = nc.scalar.copy(
                scrA[:], ident[:, 0:4]).ins
            tpl[mybir.EngineType.PE] = nc.tensor.transpose(
                cps_t[:, 0:128], ident[:], ident[:]).ins
            # SP carrier: a scratch-sem clear — same-engine program order is
            # what makes a hoisted wait actually gate the instruction
            tpl[mybir.EngineType.SP] = nc.sync.sem_clear(
                wsems[mybir.EngineType.SP]).ins
            nc._wsplit_tpl = tpl

            # ---- persistent SBUF regions -------------------------------
            U = nc.alloc_sbuf_tensor("u_all", [128, 8 * NPC], F16)
            cd = nc.alloc_sbuf_tensor("cd_all", [128, R2], F16)
            vts = nc.alloc_sbuf_tensor("vt_all", [128, R2], F16)

            # input DMAs — all on the SP queue so they never sit behind the
            # ACT evacuations; vt split in two so tile-0 compute starts early
            nc.sync.dma_start(
                bass.AP(U, 1 * NPC, [[8 * NPC, 128], [1, 3 * NPC]]),
                ux[:, :])
            nc.sync.dma_start(vts[:, 0:2048], vt[:, 0:2048])

            # cd zero-fill split across three engines (sized to finish
            # together), then one diagonal scatter DMA writes all of C:
            # cd[(g,k), (p*16+g)*64 + (G*8+t)] — 64-elem runs keep the
            # matmul rhs a single-level stride-64 AP
            chain('DVE', nc.vector.memset(cd[:, 0:2240], 0.0))
            chain('ACT', nc.scalar.memzero(cd[:, 2240:5056]))
            chain('POOL', nc.gpsimd.memset(cd[:, 5056:R2], 0.0))
            nc.sync.dma_start(vts[:, 2048:R2], vt[:, 2048:R2])
            scat_engs = (nc.sync, nc.scalar, nc.gpsimd)
            for g in range(GN):
                dst = bass.AP(cd, 8 * g * R2 + g * 64,
                              [[R2, 8], [1024, 8], [1, 64]])
                src = bass.AP(cden, g * 4096, [[512, 8], [64, 8], [1, 64]])
                scat_engs[g % 3].dma_start(dst, src)

            # u: ones slot + product slots 4..7 (p-major layout)
            chain('POOL', nc.gpsimd.memset(
                bass.AP(U, 0, [[8 * NPC, 128], [1, NPC]]), 1.0))
            uap = lambda p, d=1: bass.AP(
                U, p * NPC, [[8 * NPC, 128], [NPC, d], [1, NPC]])
            chain('DVE', nc.vector.tensor_tensor(
                uap(4), uap(1), uap(2), mybir.AluOpType.mult))
            chain('DVE', nc.vector.tensor_tensor(
                uap(5, 2), uap(1, 2),
                bass.AP(U, 3 * NPC, [[8 * NPC, 128], [0, 2], [1, NPC]]),
                mybir.AluOpType.mult))
            chain('DVE', nc.vector.tensor_tensor(
                uap(7), uap(4), uap(3), mybir.AluOpType.mult))

            out_sb = opool.tile([128, NPC], F16)

            if dbg:
                nc.sync.dma_start(d_u[:, :], bass.AP(
                    U, 0, [[8 * NPC, 128], [1, 8 * NPC]]))
                nc.sync.dma_start(d_cd[:, :], cd[:, :])
                nc.sync.dma_start(d_vt[:, :], vts[:, :])

            for t in range(NT):
                yp = y_psum.tile([128, 1024], F32, tag="yp")
                for G in range(NG):
                    rhs = bass.AP(cd, G * 8 + t,
                                  [[R2, 128], [64, 128]])
                    chain('PE', nc.tensor.matmul(
                        yp[:, G * 128:(G + 1) * 128],
                        lhsT=vts[:, t * 1024 + G * 128:t * 1024 + (G + 1) * 128],
                        rhs=rhs,
                        start=True, stop=True,
                    ))

                # evacuate PSUM -> SBUF fp16 on ACT (enables DVE 2x)
                ysb = ypool.tile([128, 1024], F16, tag="ysb")
                chain('ACT', nc.scalar.copy(ysb[:], yp[:]))
                if dbg and t == 0:
                    nc.sync.dma_start(d_ysb[:, :], ysb[:])

                # z = y * u  (all fp16 SBUF -> 2x)
                z = zpool.tile([128, 1024], F16, tag="z")
                uin = bass.AP(U, t * TN,
                              [[8 * NPC, 128], [GN, NG], [NPC, 8], [1, GN]])
                chain('DVE', nc.vector.tensor_tensor(
                    z[:].rearrange("a (G p g) -> a G p g", p=8, g=GN),
                    ysb[:].rearrange("a (G p g) -> a G p g", p=8, g=GN),
                    uin, mybir.AluOpType.mult))

                if dbg and t == 0:
                    nc.sync.dma_start(d_z[:, :], z[:])

                # binary add tree over p; L1 alternates DVE/Pool
                z4 = z[:].rearrange("a (G p g) -> a G p g", p=8, g=GN)
                l1eng, l1key = ((nc.gpsimd, 'POOL') if t % 2 == 0
                                else (nc.vector, 'DVE'))
                chain(l1key, l1eng.tensor_tensor(
                    z4[:, :, 0:4], z4[:, :, 0:4], z4[:, :, 4:8],
                    mybir.AluOpType.add))
                chain('DVE', nc.vector.tensor_tensor(
                    z4[:, :, 0:2], z4[:, :, 0:2], z4[:, :, 2:4],
                    mybir.AluOpType.add))
                chain('DVE', nc.vector.tensor_tensor(
                    out_sb[:, t * TN:(t + 1) * TN]
                        .rearrange("a (G q g) -> a G q g", q=1, g=GN),
                    z4[:, :, 0:1], z4[:, :, 1:2],
                    mybir.AluOpType.add))

                if dbg and t == 0:
                    nc.sync.dma_start(d_zt[:, :], z[:])
                    nc.sync.dma_start(d_o[:, :], out_sb[:, 0:TN])

                if t % 2 == 1:
                    nc.sync.dma_start(out[:, (t - 1) * TN:(t + 1) * TN],
                                      out_sb[:, (t - 1) * TN:(t + 1) * TN])

    _split_multiwait(nc)
    return nc


def _split_multiwait(nc):
    """Hoist extra sync waits onto same-engine carrier instructions (the
    walrus codegen gives each TPB instruction ~one wait slot)."""
    import inspect
    wsems = nc._wsplit_sems
    tpl = nc._wsplit_tpl
    clears = set(id(c) for c in nc._wsplit_clears)

    sigcache = {}

    def clone(template, engine, name, w, sem):
        ty = type(template)
        if ty not in sigcache:
            sigcache[ty] = [p for p in inspect.signature(ty).parameters
                            if p not in ("name", "engine", "sync_info",
                                         "descendants", "_kwargs")]
        kw = {}
        for p in sigcache[ty]:
            if hasattr(template, p):
                v = getattr(template, p)
                if v is not None or p in ("ins", "outs"):
                    kw[p] = v
        return ty(name=name, engine=engine,
                  sync_info=mybir.SyncInfo(on_wait=[w], on_update=[]),
                  **kw)

    for fn in nc.m.functions:
        for blk in fn.blocks:
            head, out = [], []
            changed = False
            for ins in blk.instructions:
                if id(ins) in clears:
                    head.append(ins)
                    changed = True
                    continue
                si = getattr(ins, "sync_info", None)
                waits = list(si.on_wait) if si is not None else []
                if len(waits) > 1:
                    changed = True
                    eng = ins.engine
                    ceng = eng if eng in tpl else mybir.EngineType.Pool
                    # carriers are transpose clones on PE and would clobber
                    # the stationary between an Ldweights/Matmult pair —
                    # insert them BEFORE the matmul's Ldweights
                    pos = len(out)
                    if (type(ins).__name__ == "InstMatmult" and out
                            and type(out[-1]).__name__ == "InstLdweights"
                            and out[-1].engine == ins.engine):
                        pos -= 1
                    carriers = [clone(tpl[ceng], ceng,
                                      f"{ins.name}-w{i}", w, wsems[ceng])
                                for i, w in enumerate(waits[:-1])]
                    out[pos:pos] = carriers
                    ins.sync_info = mybir.SyncInfo(
                        on_wait=[waits[-1]], on_update=list(si.on_update))
                out.append(ins)
            if changed:
                blk.instructions = head + out


# ---------------------------------------------------------------- host side

# slot order [1, a, b, c, ab, ac, bc, abc] -> monomial bitmask (bit0=a,...)
SLOT2MON = np.array([0, 1, 2, 4, 3, 5, 6, 7])


def _monomial_C(tables: np.ndarray) -> np.ndarray:
    """tables (N, 64) -> C (N, 8, 8) fp32 in slot order: C[n, p, k]."""
    c = np.asarray(tables, np.float64).reshape(-1, 2, 2, 2, 2, 2, 2)
    for ax in range(1, 7):
        lo = np.take(c, 0, axis=ax)
        hi = np.take(c, 1, axis=ax)
        c = np.stack([lo, hi - lo], axis=ax)
    # axes (n, m5, m4, m3, m2, m1, m0): flat index m5*32+...+m0
    cm = c.reshape(-1, 64)
    flat = np.zeros((8, 8), np.int64)
    for jm in range(8):
        for km in range(8):
            m0, m1, m2 = jm & 1, (jm >> 1) & 1, (jm >> 2) & 1
            m3, m4, m5 = km & 1, (km >> 1) & 1, (km >> 2) & 1
            flat[jm, km] = m5 * 32 + m4 * 16 + m3 * 8 + m2 * 4 + m1 * 2 + m0
    idx = flat[SLOT2MON][:, SLOT2MON]   # idx[p, k], slot-ordered
    return cm[:, idx].astype(np.float32)  # (N, 8, 8)


def _v_monomials(xv: np.ndarray) -> np.ndarray:
    """xv (..., 3) -> (..., 8) slot-order monomials [1,a,b,c,ab,ac,bc,abc]."""
    a, b, c = xv[..., 0], xv[..., 1], xv[..., 2]
    one = np.ones_like(a)
    return np.stack([one, a, b, c, a * b, a * c, b * c, a * b * c], axis=-1)


def make_in_maps(x: np.ndarray, tables: np.ndarray):
    x = np.clip(np.asarray(x, np.float32), 0.0, 1.0)
    C = _monomial_C(np.asarray(tables, np.float32))  # (N, 8, 8)
    in_maps = []
    for core in range(NCORES):
        sl = slice(core * NPC, (core + 1) * NPC)
        xs = x[:, sl, :]                            # (B, 1024, 6)

        # ux: [b, (j, nl)] j-major raw u-vars x0..x2
        uxc = np.ascontiguousarray(
            xs[:, :, 0:3].transpose(0, 2, 1).reshape(B, 3 * NPC)
        ).astype(np.float16)

        # vt: [8g+k, t*1024 + G*128 + b] = v_k[b, node t*128+G*16+g]
        vmon = _v_monomials(xs[:, :, 3:6]).astype(np.float16)  # (B,1024,8)
        vm = vmon.reshape(B, NT, NG, GN, 8)          # (b, t, G, g, k)
        vtc = np.ascontiguousarray(
            vm.transpose(3, 4, 1, 2, 0)              # (g, k, t, G, b)
            .reshape(128, R2))

        # cden: (g, k, p, G, t)
        Cc = C[sl].reshape(NT, NG, GN, 8, 8)         # (t, G, g, p, k)
        cdenc = np.ascontiguousarray(
            Cc.transpose(2, 4, 3, 1, 0)              # (g, k, p, G, t)
        ).reshape(GN * 8 * 512).astype(np.float16)

        in_maps.append({"ux": uxc, "vt": vtc, "cden": cdenc})
    return in_maps


_NC_CACHE = None


def _get_nc():
    global _NC_CACHE
    if _NC_CACHE is None:
        _NC_CACHE = build_nc()
    return _NC_CACHE


def kernel(x: np.ndarray, tables: np.ndarray, _trace: bool = False):
    nc = _get_nc()
    in_maps = make_in_maps(x, tables)
    res = bass_utils.run_bass_kernel_spmd(
        nc, in_maps, core_ids=list(range(NCORES)), trace=_trace,
    )
    out = np.concatenate(
        [r["out"].astype(np.float32) for r in res.results], axis=1)
    if _trace:
        return out, res
    return out
